# revision 1
# baseline (speedup 1.0000x reference)
"""Bar-level attention Trainium2 kernel (8 NeuronCores, head-parallel).

Contract: kernel(**inputs) takes the FULL inputs from setup_inputs() and
returns the FULL [1, 2048, 512] float32 output.

Strategy (one head per core, 8 heads / 8 cores):
  - Host: transpose hidden -> XT [512, 2048]; slice + transpose per-head
    weights; fold the 1/sqrt(dh) score scale into Wq/bq; compute
    g = sigmoid(gate[h]) on host and ship as replicated [128,1] columns.
  - Device (per core, all fp32):
      XT -> Q^T, K^T [64, 2048] and V [2048, 65] (col 64 = ones).
      For each 1024-wide query half and each 128-row key chunk:
        S^T = K_chunk @ Q^T  (keys on partitions, queries on free axis)
        E = exp(S^T)         (no max subtraction: scores ~ N(0,1))
        global unnorm AV  += V_chunk~.T @ E        -> [65, 1024] PSUM
        local  unnorm AV  += per-bar diagonal-block matmuls (bar_positions
                             are sorted -> blocks are contiguous; block
                             spans are baked in at build time)
        Row 64 of each AV accumulator is the softmax denominator (ones col).
      Final: project both AV results through Wo_h slice, rescale rows by
      g/l_local and (1-g)/l_global, add -> partial output [2048, 512].
  - Host: sum the 8 partial outputs (output projection is sharded over the
    contraction dim) + bo -> [1, 2048, 512].

The global-attention additive bias in the reference is per-query (constant
across keys), and softmax is shift-invariant per row, so it drops out
exactly; global attention is plain dense softmax attention.
"""

import numpy as np

S = 2048
D = 512
H = 8
DH = 64
SCALE = 1.0 / np.sqrt(DH)
NCHUNK = S // 128      # 16 key chunks of 128
NHALF = 2              # query halves of 1024
QHALF = S // NHALF


def _legalize_waits(nc, mybir):
    """This walrus codegen accepts at most ONE sync wait per instruction.
    Split any instruction carrying N>1 waits into N-1 preceding single-wait
    NoOps on the same engine (waits execute in order on the sequencer)."""
    ctr = 0
    for f in nc.m.functions:
        for b in f.blocks:
            insts = b.instructions
            if not any(i.sync_info and len(i.sync_info.on_wait) > 1 for i in insts):
                continue
            new = []
            for ins in insts:
                si = ins.sync_info
                if si is not None and len(si.on_wait) > 1:
                    waits = list(si.on_wait)
                    for w in waits[:-1]:
                        ctr += 1
                        nop = mybir.InstNoOp(name=f"waitsplit-{ctr}", engine=ins.engine)
                        nop.sync_info = mybir.SyncInfo(on_wait=[w], on_update=[])
                        new.append(nop)
                    ins.sync_info = mybir.SyncInfo(
                        on_wait=[waits[-1]], on_update=list(si.on_update)
                    )
                new.append(ins)
            insts.clear()
            insts.extend(new)
    return ctr


def _bar_bounds(bp):
    """bp: sorted int array [S] -> list of (start, end) per bar."""
    change = np.nonzero(np.diff(bp))[0] + 1
    starts = np.concatenate([[0], change])
    ends = np.concatenate([change, [len(bp)]])
    return list(zip(starts.tolist(), ends.tolist()))


def _build(bars):
    import concourse.bass as bass
    import concourse.tile as tile
    import concourse.mybir as mybir

    dt = mybir.dt
    AF = mybir.ActivationFunctionType
    OP = mybir.AluOpType
    f32 = dt.float32
    f32r = dt.float32r

    def F(ap):
        # view a float32r tile as plain fp32 (for the small local-AV matmuls
        # and DVE ops; f32r tiles hold rounded fp32 bits)
        return ap.bitcast(f32)

    nc = bass.Bass()
    xt_d = nc.dram_tensor("xt", [D, S], f32r, kind="ExternalInput")
    # wpack: 4 chunks of [128, 192]: cols 0:64 WqT(scaled), 64:128 WkT, 128:192 WvT
    wpack_d = nc.dram_tensor("wpack", [D, 192], f32r, kind="ExternalInput")
    wot_d = nc.dram_tensor("wot", [DH, D], f32r, kind="ExternalInput")
    # smalls [128, 8]: col0 bq/8 (rows 0:64), col1 bk, col2 bv, col3 g,
    # col4 1-g, col5 ones
    smalls_d = nc.dram_tensor("smalls", [128, 8], f32, kind="ExternalInput")
    zeros_d = nc.dram_tensor("zeros", [128, 512], f32r, kind="ExternalInput")
    # mask bands: chunk c occupies cols [c*512, c*512+w_c); m[kk, j] = 1 iff
    # bar(c*128+kk) == bar(blo_c + j)
    mask_d = nc.dram_tensor("maskband", [128, NCHUNK * 512], f32, kind="ExternalInput")
    out_d = nc.dram_tensor("out_partial", [S, D], f32, kind="ExternalOutput")

    # per-chunk global band [blo_c, bhi_c): union of bars intersecting chunk
    band = []
    for c in range(NCHUNK):
        klo, khi = c * 128, (c + 1) * 128
        bs = [b for b in bars if b[1] > klo and b[0] < khi]
        band.append((bs[0][0], bs[-1][1]))
        assert band[-1][1] - band[-1][0] <= 512

    with tile.TileContext(nc, pool_alloc_mode="queue") as tc:
        with (
            tc.tile_pool(name="persist", bufs=1) as p_keep,
            tc.tile_pool(name="outbuf", bufs=1) as p_out,
        ):
            qt = p_keep.tile([DH, S], f32r, tag="qt")
            kt = p_keep.tile([DH, S], f32r, tag="kt")
            zeros = p_keep.tile([128, 512], f32r, tag="zeros")
            vt = [p_keep.tile([128, DH + 1], f32r, tag=f"vt{c}", name=f"vt{c}") for c in range(NCHUNK)]
            smalls = p_keep.tile([128, 8], f32, tag="smalls")
            wot = p_keep.tile([DH, D], f32r, tag="wot")
            maskt = p_keep.tile([128, NCHUNK * 512], f32, tag="maskt")
            outbuf = p_out.tile([128, NCHUNK * D], f32, tag="outbuf")

            # ---------------- projections ----------------
            with (
                tc.tile_pool(name="inp", bufs=1) as p_in,
                tc.tile_pool(name="pj", bufs=2, space="PSUM") as p_pj,
                tc.tile_pool(name="pv", bufs=2, space="PSUM") as p_pv,
            ):
                xts = [p_in.tile([128, S], f32r, tag=f"xt{i}", name=f"xts{i}") for i in range(4)]
                wps = [p_in.tile([128, 192], f32r, tag=f"wp{i}", name=f"wps{i}") for i in range(4)]
                nc.sync.dma_start(smalls[:], smalls_d[:])
                nc.sync.dma_start(zeros[:], zeros_d[:])
                for i in range(4):
                    nc.sync.dma_start(
                        wps[i][:], wpack_d[i * 128 : (i + 1) * 128, :]
                    )
                # two column panels per tile: the first QT/KT/V matmul groups
                # unlock after ~2MB instead of the full 4MB. The 4MB mask
                # band is deferred: the shared SDMA engines serialize
                # transfers, and the mask isn't consumed until the first
                # local-AV (~25us in).
                for n in range(2):
                    for i in range(4):
                        nc.sync.dma_start(
                            xts[i][:, n * 1024 : (n + 1) * 1024],
                            xt_d[i * 128 : (i + 1) * 128, n * 1024 : (n + 1) * 1024],
                        )
                # mask on the same (sync) queue: a gpsimd-issued DMA would
                # start immediately (Pool engine idle) and hog the shared
                # SDMA engines ahead of the XT panels
                nc.sync.dma_start(maskt[:], mask_d[:])
                nc.sync.dma_start(wot[:], wot_d[:])

                # Q^T and K^T: [64, 2048] in 1024-halves through 2 psum bufs
                for which, dest, wcol, bcol in ((0, qt, 0, 0), (1, kt, 64, 1)):
                    for hq in range(NHALF):
                        ps = p_pj.tile([DH, QHALF], f32, tag="pj")
                        for n in range(QHALF // 512):
                            for kc in range(4):
                                nc.tensor.matmul(
                                    ps[:, n * 512 : (n + 1) * 512],
                                    (wps[kc][:, wcol : wcol + 64]),
                                    (xts[kc][
                                        :,
                                        hq * QHALF + n * 512 : hq * QHALF + (n + 1) * 512,
                                    ]),
                                    start=(kc == 0),
                                    stop=(kc == 3),
                                )
                        nc.scalar.activation(
                            dest[:, hq * QHALF : (hq + 1) * QHALF],
                            ps[:],
                            AF.Identity,
                            bias=smalls[0:DH, bcol : bcol + 1],
                        )

                # V in natural [k, dh] layout, chunk by chunk; col 64 = 1.0
                for c in range(NCHUNK):
                    ps = p_pv.tile([128, DH], f32, tag="pv")
                    for kc in range(4):
                        nc.tensor.matmul(
                            ps[:],
                            (xts[kc][:, c * 128 : (c + 1) * 128]),
                            (wps[kc][:, 128:192]),
                            start=(kc == 0),
                            stop=(kc == 3),
                        )
                    nc.scalar.activation(
                        vt[c][:, 0:DH],
                        ps[:],
                        AF.Identity,
                        bias=smalls[:, 2:3],
                    )
                    nc.scalar.copy(vt[c][:, DH : DH + 1], smalls[:, 5:6])

            # ---------------- attention ----------------
            # per-(half, chunk) bar pieces baked from bar_positions
            ogs = []
            ols = []
            with tc.tile_pool(name="avout", bufs=1) as p_av:
                l2l = p_av.tile([128, NCHUNK], f32r, tag="l2l")
                l2g = p_av.tile([128, NCHUNK], f32r, tag="l2g")
                r2l = p_av.tile([128, NCHUNK], f32, tag="r2l")
                r2g = p_av.tile([128, NCHUNK], f32, tag="r2g")
                with (
                    tc.tile_pool(name="ps", bufs=2, space="PSUM") as p_s,
                    tc.tile_pool(name="pog", bufs=1, space="PSUM") as p_og,
                    tc.tile_pool(name="pol", bufs=1, space="PSUM") as p_ol,
                    tc.tile_pool(name="pe", bufs=3) as p_e,
                    tc.tile_pool(name="pel", bufs=2) as p_el,
                ):
                  for hq in range(NHALF):
                    og_sb = p_av.tile([DH + 1, QHALF], f32r, tag=f"og{hq}", name=f"ogsb{hq}")
                    ol_sb = p_av.tile([DH + 1, QHALF], f32r, tag=f"ol{hq}", name=f"olsb{hq}")
                    ogs.append(og_sb)
                    ols.append(ol_sb)
                    if True:
                        og = p_og.tile([DH + 1, QHALF], f32, tag="og", name=f"og{hq}")
                        ol = p_ol.tile([DH + 1, QHALF], f32, tag="ol", name=f"ol{hq}")
                        # HW: start=True clears has_written for the WHOLE
                        # bank, so interleaved per-region accumulation groups
                        # corrupt each other. Zero-init ol once with a full
                        # width start=True matmul against zeros, then every
                        # local piece accumulates with start=False.
                        for n in range(QHALF // 512):
                            nc.tensor.matmul(
                                ol[:, n * 512 : (n + 1) * 512],
                                vt[0][:],
                                zeros[:],
                                start=True,
                                stop=False,
                                skip_group_check=True,
                            )
                        for c in range(NCHUNK):
                            sc = p_s.tile([128, QHALF], f32, tag="s")
                            for n in range(QHALF // 512):
                                nc.tensor.matmul(
                                    sc[:, n * 512 : (n + 1) * 512],
                                    (kt[:, c * 128 : (c + 1) * 128]),
                                    (qt[
                                        :,
                                        hq * QHALF + n * 512 : hq * QHALF + (n + 1) * 512,
                                    ]),
                                    start=True,
                                    stop=True,
                                )
                            ec = p_e.tile([128, QHALF], f32r, tag="e")
                            nc.scalar.activation(ec[:], sc[:], AF.Exp)
                            # global AV accumulation
                            for n in range(QHALF // 512):
                                nc.tensor.matmul(
                                    og[:, n * 512 : (n + 1) * 512],
                                    (vt[c][:]),
                                    (ec[:, n * 512 : (n + 1) * 512]),
                                    start=(c == 0),
                                    stop=(c == NCHUNK - 1),
                                )
                            # local AV: masked band of E (bars are contiguous
                            # diagonal blocks); matmul base partitions must be
                            # 0/32/64, so zero-pad a full-128-row band copy.
                            klo, khi = c * 128, (c + 1) * 128
                            qlo, qhi = hq * QHALF, (hq + 1) * QHALF
                            pieces = []  # (qs, qe, rlo, rhi, start, stop)
                            for (s_b, e_b) in bars:
                                if e_b <= klo or s_b >= khi:
                                    continue
                                qs = max(s_b, qlo)
                                qe = min(e_b, qhi)
                                if qs >= qe:
                                    continue
                                pieces.append(
                                    (
                                        qs,
                                        qe,
                                        max(s_b, klo) - klo,
                                        min(e_b, khi) - klo,
                                        s_b >= klo,
                                        e_b <= khi,
                                    )
                                )
                            if pieces:
                                blo, bhi = band[c]
                                hs = pieces[0][0]   # half-clipped band start
                                he = pieces[-1][1]
                                w = he - hs
                                el = p_el.tile([128, 512], f32, tag="el", name="el")
                                nc.vector.tensor_mul(
                                    el[:, 0:w],
                                    F(ec[:, hs - qlo : he - qlo]),
                                    maskt[:, c * 512 + (hs - blo) : c * 512 + (he - blo)],
                                )
                                # matmul runs: merge adjacent pieces with same
                                # flags, split at 512-col psum bank boundaries
                                runs = []
                                for (qs, qe, _, _, st, sp) in pieces:
                                    if runs and runs[-1][2] == st and runs[-1][3] == sp and runs[-1][1] == qs:
                                        runs[-1][1] = qe
                                    else:
                                        runs.append([qs, qe, st, sp])
                                for (qs, qe, st, sp) in runs:
                                    a = qs
                                    while a < qe:
                                        b_ = min(qe, ((a - qlo) // 512 + 1) * 512 + qlo)
                                        nc.tensor.matmul(
                                            ol[:, a - qlo : b_ - qlo],
                                            F(vt[c][:]),
                                            el[:, a - hs : b_ - hs],
                                            start=False,
                                            stop=False,
                                            skip_group_check=True,
                                        )
                                        a = b_
                        # close the ol accumulation group (adds zeros)
                        for n in range(QHALF // 512):
                            nc.tensor.matmul(
                                ol[:, n * 512 : (n + 1) * 512],
                                vt[0][:],
                                zeros[:],
                                start=False,
                                stop=True,
                                skip_group_check=True,
                            )
                        nc.scalar.copy(og_sb[:], og[:])
                        nc.scalar.copy(ol_sb[:], ol[:])
                    # denominator rows -> [128, 8] reshape, natural order:
                    # l2[p, hq*8+jj] = l_half[p*8+jj]; issued per half so
                    # half0's transfers hide under half1's attention
                    j0 = hq * (NCHUNK // NHALF)
                    nc.sync.dma_start(
                        l2l[:, j0 : j0 + NCHUNK // NHALF], ol_sb[DH : DH + 1, :]
                    )
                    nc.sync.dma_start(
                        l2g[:, j0 : j0 + NCHUNK // NHALF], og_sb[DH : DH + 1, :]
                    )

                # ---------------- denominators + recip ----------------
                if True:
                    nc.vector.reciprocal(r2l[:], F(l2l[:]))
                    nc.vector.reciprocal(r2g[:], F(l2g[:]))
                    # fold gate: r_l *= g, r_g *= (1-g)
                    nc.vector.tensor_scalar_mul(r2l[:], r2l[:], smalls[:, 3:4])
                    nc.vector.tensor_scalar_mul(r2g[:], r2g[:], smalls[:, 4:5])

                    # ---------------- output projection + combine ----------
                    with (
                        tc.tile_pool(name="pp", bufs=4, space="PSUM") as p_pp,
                        tc.tile_pool(name="t1", bufs=2) as p_t1,
                    ):
                        for j in range(NCHUNK):
                            hq = j // (NCHUNK // NHALF)
                            jj = j % (NCHUNK // NHALF)
                            # interleaved query chunk: cols jj, jj+8, ...
                            lp = p_pp.tile([128, D], f32, tag="pp")
                            nc.tensor.matmul(
                                lp[:],
                                (ols[hq][0:DH, jj : QHALF : NCHUNK // NHALF]),
                                (wot[:]),
                                start=True,
                                stop=True,
                            )
                            gp = p_pp.tile([128, D], f32, tag="pp")
                            nc.tensor.matmul(
                                gp[:],
                                (ogs[hq][0:DH, jj : QHALF : NCHUNK // NHALF]),
                                (wot[:]),
                                start=True,
                                stop=True,
                            )
                            t1 = p_t1.tile([128, D], f32, tag="t1")
                            # t1 = lp * r_l[q]  (per-partition scale)
                            nc.vector.tensor_scalar_mul(
                                t1[:], lp[:], r2l[:, j : j + 1]
                            )
                            # out = gp * r_g[q] + t1  (DVE fused)
                            nc.vector.scalar_tensor_tensor(
                                outbuf[:, j * D : (j + 1) * D],
                                gp[:],
                                r2g[:, j : j + 1],
                                t1[:],
                                OP.mult,
                                OP.add,
                            )
                            grp = {3: (0, 4), 7: (4, 4), 11: (8, 4),
                                   13: (12, 2), 14: (14, 1), 15: (15, 1)}.get(j)
                            if grp:
                                c0, ng = grp
                                hq_ = c0 // (NCHUNK // NHALF)
                                jj0 = c0 % (NCHUNK // NHALF)
                                dst = out_d[
                                    hq_ * QHALF : (hq_ + 1) * QHALF, :
                                ].rearrange("(p j) c -> p j c", j=NCHUNK // NHALF)[
                                    :, jj0 : jj0 + ng, :
                                ]
                                srcb = outbuf[
                                    :, c0 * D : (c0 + ng) * D
                                ].rearrange("p (j c) -> p j c", j=ng)
                                nc.sync.dma_start(dst, srcb)

    _legalize_waits(nc, mybir)
    return nc


_CACHE = {}


def _get_built(bar_key, bars):
    if bar_key not in _CACHE:
        _CACHE[bar_key] = _build(bars)
    return _CACHE[bar_key]


def _np_reference(hidden_states, bar_positions, attention_mask, Wq, bq, Wk, bk,
                  Wv, bv, Wo, bo, bar_emb, gate):
    """Plain numpy fallback (only used if inputs violate baked assumptions)."""
    B, S_, _ = hidden_states.shape
    x = hidden_states.astype(np.float64)
    q = (x @ Wq.T + bq).reshape(B, S_, H, DH).transpose(0, 2, 1, 3)
    k = (x @ Wk.T + bk).reshape(B, S_, H, DH).transpose(0, 2, 1, 3)
    v = (x @ Wv.T + bv).reshape(B, S_, H, DH).transpose(0, 2, 1, 3)
    scores = np.einsum("bhqd,bhkd->bhqk", q, k) * SCALE
    pad = attention_mask[:, None, None, :]
    bar_mask = (bar_positions[:, :, None] == bar_positions[:, None, :])[:, None]
    NEG = -np.inf

    def softmax(s):
        s = s - s.max(-1, keepdims=True)
        e = np.exp(s)
        return e / e.sum(-1, keepdims=True)

    local = softmax(np.where(bar_mask & pad, scores, NEG))
    emb = bar_emb[np.asarray(bar_positions) % bar_emb.shape[0]]
    bias = np.sum(emb * emb, axis=-1)
    glob = softmax(np.where(pad, scores + bias[:, None, :, None], NEG))
    la = np.einsum("bhqk,bhkd->bhqd", local, v)
    ga = np.einsum("bhqk,bhkd->bhqd", glob, v)
    g = 1.0 / (1.0 + np.exp(-gate))[None, :, None, None]
    comb = g * la + (1.0 - g) * ga
    out = comb.transpose(0, 2, 1, 3).reshape(B, S_, H * DH)
    return (out @ Wo.T + bo).astype(np.float32)


def kernel(**inputs):
    hidden_states = np.asarray(inputs["hidden_states"], dtype=np.float32)
    bar_positions = np.asarray(inputs["bar_positions"])
    attention_mask = np.asarray(inputs["attention_mask"])
    Wq = np.asarray(inputs["Wq"], dtype=np.float32)
    bq = np.asarray(inputs["bq"], dtype=np.float32)
    Wk = np.asarray(inputs["Wk"], dtype=np.float32)
    bk = np.asarray(inputs["bk"], dtype=np.float32)
    Wv = np.asarray(inputs["Wv"], dtype=np.float32)
    bv = np.asarray(inputs["bv"], dtype=np.float32)
    Wo = np.asarray(inputs["Wo"], dtype=np.float32)
    bo = np.asarray(inputs["bo"], dtype=np.float32)
    gate = np.asarray(inputs["gate"], dtype=np.float32)

    bp = bar_positions[0].astype(np.int64)
    if (
        hidden_states.shape != (1, S, D)
        or not bool(attention_mask.all())
        or not bool((np.diff(bp) >= 0).all())
    ):
        return _np_reference(
            hidden_states, bar_positions, attention_mask, Wq, bq, Wk, bk,
            Wv, bv, Wo, bo, np.asarray(inputs["bar_emb"], dtype=np.float32), gate,
        )

    bars = _bar_bounds(bp)
    nc = _get_built(bp.tobytes(), bars)

    # mask bands (same for every core)
    maskband = np.zeros((128, NCHUNK * 512), dtype=np.float32)
    for c in range(NCHUNK):
        klo, khi = c * 128, (c + 1) * 128
        bs = [b for b in bars if b[1] > klo and b[0] < khi]
        blo = bs[0][0]
        eq = (bp[klo:khi, None] == bp[None, blo : bs[-1][1]])
        maskband[:, c * 512 : c * 512 + eq.shape[1]] = eq.astype(np.float32)

    xt = np.ascontiguousarray(hidden_states[0].T)  # [512, 2048]
    g = 1.0 / (1.0 + np.exp(-gate.astype(np.float64)))  # sigmoid, [H]
    in_maps = []
    for h in range(H):
        sl = slice(h * DH, (h + 1) * DH)
        wpack = np.empty((D, 192), dtype=np.float32)
        wpack[:, 0:64] = Wq[sl, :].T * np.float32(SCALE)
        wpack[:, 64:128] = Wk[sl, :].T
        wpack[:, 128:192] = Wv[sl, :].T
        wot = np.ascontiguousarray(Wo[:, sl].T)  # [64, 512]
        smalls = np.zeros((128, 8), dtype=np.float32)
        smalls[0:DH, 0] = bq[sl] * np.float32(SCALE)
        smalls[0:DH, 1] = bk[sl]
        smalls[0:DH, 2] = bv[sl]
        smalls[:, 3] = np.float32(g[h])
        smalls[:, 5] = 1.0
        smalls[:, 4] = np.float32(1.0 - g[h])
        in_maps.append(
            {"xt": xt, "wpack": wpack, "wot": wot, "smalls": smalls,
             "maskband": maskband, "zeros": np.zeros((128, 512), np.float32)}
        )

    res = _run_spmd(nc, in_maps)
    out = np.zeros((S, D), dtype=np.float32)
    for h in range(H):
        out += res.results[h]["out_partial"]
    out += bo
    return out.reshape(1, S, D)


def _run_spmd(nc, in_maps, **kw):
    from concourse.bass_utils import run_bass_kernel_spmd

    return run_bass_kernel_spmd(nc, in_maps, list(range(H)), **kw)



# revision 6
# speedup vs baseline: 1.0590x; 1.0590x over previous
"""Bar-level attention Trainium2 kernel (8 NeuronCores, head-parallel).

Contract: kernel(**inputs) takes the FULL inputs from setup_inputs() and
returns the FULL [1, 2048, 512] float32 output.

Strategy (one head per core, 8 heads / 8 cores), v2 layout:
  - Host: XT [512, 2048] bf16; per-head packed weights wqk = [WqT*scale |
    WkT] bf16, wv = WvT bf16, wot = WoT slice bf16; gate folded into
    per-partition scalars; bq/bk are zero in setup_inputs (guarded); bv/bo
    folded into the host-side bias add (exact: softmax rows sum to 1).
  - Device (per core):
      QK^T [128, 2048]: rows 0:64 = Q^T (scale folded), 64:128 = K^T; f32r.
      V [k, 65] bf16 per 128-key chunk (col 64 = ones -> denominators).
      Attention per 1024-query half, per 128-key chunk:
        S^T = K_chunk @ Q^T -> psum [128, 1024] (keys on partitions)
        E = exp(S^T) -> bf16 (no max subtraction: scores ~ N(0,1))
        global AV: out[q_sub 128, 65] += E_sub^T @ V  (full 128-part
          contraction; 8 query subtiles/half; accumulators for G and L
          packed 3-subtiles-per-psum-bank, zero-initialized by one
          start=True matmul against a zeroed tile so all real AV matmuls
          are order-independent start=False accumulations)
        local AV: same shape, with E masked to same-bar keys (mask band
          shipped from host, 128-aligned per chunk).
      Readout per half: 1/l via strided reciprocal over packed denominator
      columns, gate-fold, rescale+combine -> comb [q, 64] bf16; DMA-XBAR
      transpose -> comb^T; single output projection [q 128, 512] per
      subtile; psum -> sbuf copy (DVE, Act for the tail half) -> DMA out.
  - Host: sum the 8 partial outputs + bo + Wo@bv.

The global-attention additive bias in the reference is per-query and
softmax is shift-invariant per row, so it drops out exactly.
"""

import numpy as np

S = 2048
D = 512
H = 8
DH = 64
SCALE = 1.0 / np.sqrt(DH)
NCHUNK = S // 128       # 16 key chunks of 128
NHALF = 2               # query halves of 1024
QHALF = S // NHALF
NSUB = QHALF // 128     # 8 query subtiles per half
MB = 640                # aligned mask band width per chunk
# accumulator packing: 3 query-subtiles per psum bank, [G 64|lg|L 64|ll]
ACC_GROUPS = [(0, 1, 2), (3, 4, 5), (6, 7)]


def _legalize_waits(nc, mybir):
    """This walrus codegen accepts at most ONE sync wait per instruction.
    Split any instruction carrying N>1 waits into N-1 preceding single-wait
    NoOps on the same engine (waits execute in order on the sequencer)."""
    ctr = 0
    for f in nc.m.functions:
        for b in f.blocks:
            insts = b.instructions
            if not any(i.sync_info and len(i.sync_info.on_wait) > 1 for i in insts):
                continue
            new = []
            for ins in insts:
                si = ins.sync_info
                if si is not None and len(si.on_wait) > 1:
                    waits = list(si.on_wait)
                    for w in waits[:-1]:
                        ctr += 1
                        nop = mybir.InstNoOp(name=f"waitsplit-{ctr}", engine=ins.engine)
                        nop.sync_info = mybir.SyncInfo(on_wait=[w], on_update=[])
                        new.append(nop)
                    ins.sync_info = mybir.SyncInfo(
                        on_wait=[waits[-1]], on_update=list(si.on_update)
                    )
                new.append(ins)
            insts.clear()
            insts.extend(new)
    return ctr


def _bar_bounds(bp):
    """bp: sorted int array [S] -> list of (start, end) per bar."""
    change = np.nonzero(np.diff(bp))[0] + 1
    starts = np.concatenate([[0], change])
    ends = np.concatenate([change, [len(bp)]])
    return list(zip(starts.tolist(), ends.tolist()))


def _abands(bars):
    """Per chunk: 128-aligned query band of bars intersecting the chunk."""
    ab = []
    for c in range(NCHUNK):
        klo, khi = c * 128, (c + 1) * 128
        bs = [b for b in bars if b[1] > klo and b[0] < khi]
        blo, bhi = bs[0][0], bs[-1][1]
        alo = (blo // 128) * 128
        ahi = ((bhi + 127) // 128) * 128
        assert ahi - alo <= MB
        ab.append((alo, ahi))
    return ab


def _build(bars):
    import concourse.bass as bass
    import concourse.tile as tile
    import concourse.mybir as mybir

    dt = mybir.dt
    AF = mybir.ActivationFunctionType
    OP = mybir.AluOpType
    f32 = dt.float32
    f32r = dt.float32r
    bf16 = dt.bfloat16

    nc = bass.Bass()
    xt_d = nc.dram_tensor("xt", [D, S], bf16, kind="ExternalInput")
    wqk_d = nc.dram_tensor("wqk", [D, 128], bf16, kind="ExternalInput")
    wv_d = nc.dram_tensor("wv", [D, DH], bf16, kind="ExternalInput")
    wot_d = nc.dram_tensor("wot", [DH, D], bf16, kind="ExternalInput")
    # smalls [128, 8] f32: cols 0..5 = [1-g, g, 1-g, g, 1-g, g]
    smalls_d = nc.dram_tensor("smalls", [128, 8], f32, kind="ExternalInput")
    # mask bands: chunk c at cols [c*MB, c*MB + (ahi_c - alo_c));
    # m[kk, j] = 1 iff bar(c*128+kk) == bar(alo_c + j)
    mask_d = nc.dram_tensor("maskband", [128, NCHUNK * MB], bf16, kind="ExternalInput")
    out_d = nc.dram_tensor("out_partial", [S, D], f32, kind="ExternalOutput")

    aband = _abands(bars)
    # lAV schedule: for (hq, c) the clipped aligned band and its subtiles
    lav = {}  # (hq, c) -> (s_lo, s_hi, [global subtile indices])
    sub_chunks = {}  # global subtile -> list of chunks writing its L region
    for hq in range(NHALF):
        qlo, qhi = hq * QHALF, (hq + 1) * QHALF
        for c in range(NCHUNK):
            alo, ahi = aband[c]
            s_lo, s_hi = max(alo, qlo), min(ahi, qhi)
            if s_lo >= s_hi:
                continue
            subs = list(range(s_lo // 128, s_hi // 128))
            lav[(hq, c)] = (s_lo, s_hi, subs)
            for sg in subs:
                sub_chunks.setdefault(sg, []).append(c)
    assert all(sg in sub_chunks for sg in range(NHALF * NSUB))

    def acc_region(acc_tiles, s_loc):
        """accumulator tile + col offsets for local subtile s_loc (0..7)."""
        for t, grp in enumerate(ACC_GROUPS):
            if s_loc in grp:
                k = grp.index(s_loc)
                return acc_tiles[t], k * 130
        raise AssertionError

    with tile.TileContext(nc, pool_alloc_mode="queue") as tc:
        with (
            tc.tile_pool(name="persist", bufs=1) as p_keep,
            tc.tile_pool(name="outb", bufs=1) as p_out,
        ):
            qt = p_keep.tile([DH, S], f32r, tag="qt")
            kt = p_keep.tile([DH, S], f32r, tag="kt")
            vt = [p_keep.tile([128, DH + 1], bf16, tag=f"vt{c}", name=f"vt{c}")
                  for c in range(NCHUNK)]
            maskt = p_keep.tile([128, NCHUNK * MB], bf16, tag="maskt")
            smalls = p_keep.tile([128, 8], f32, tag="smalls")
            wot = p_keep.tile([DH, D], bf16, tag="wot")
            zl = p_keep.tile([128, 390], bf16, tag="zl")
            outbuf = p_out.tile([128, NHALF * NSUB * D], f32, tag="outbuf")

            # ---------------- input DMAs + projections ----------------
            with (
                tc.tile_pool(name="inp", bufs=1) as p_in,
                tc.tile_pool(name="pj", bufs=2, space="PSUM") as p_pj,
                tc.tile_pool(name="pv", bufs=2, space="PSUM") as p_pv,
            ):
                xts = [p_in.tile([128, S], bf16, tag=f"xt{i}", name=f"xts{i}")
                       for i in range(4)]
                wqks = [p_in.tile([128, 128], bf16, tag=f"wqk{i}", name=f"wqks{i}")
                        for i in range(4)]
                wvs = [p_in.tile([128, DH], bf16, tag=f"wv{i}", name=f"wvs{i}")
                       for i in range(4)]
                nc.sync.dma_start(smalls[:], smalls_d[:])
                for i in range(4):
                    nc.sync.dma_start(wqks[i][:], wqk_d[i * 128:(i + 1) * 128, :])
                for i in range(4):
                    nc.sync.dma_start(wvs[i][:], wv_d[i * 128:(i + 1) * 128, :])
                nc.sync.dma_start(wot[:], wot_d[:])
                # xt panels and mask chunks, interleaved so mask c0 lands
                # before the first local-AV needs it (single DMA queue
                # serializes in emission order)
                def xt_panel(p):
                    for kc in range(4):
                        nc.sync.dma_start(
                            xts[kc][:, p * 512:(p + 1) * 512],
                            xt_d[kc * 128:(kc + 1) * 128, p * 512:(p + 1) * 512],
                        )
                def mask_chunks(cs):
                    for c in cs:
                        alo, ahi = aband[c]
                        w = ahi - alo
                        nc.sync.dma_start(
                            maskt[:, c * MB:c * MB + w],
                            mask_d[:, c * MB:c * MB + w],
                        )
                xt_panel(0)
                xt_panel(1)
                mask_chunks(range(0, 4))
                xt_panel(2)
                mask_chunks(range(4, 9))
                xt_panel(3)
                mask_chunks(range(9, 16))

                # zeros + ones-column init (Pool engine, no deps)
                nc.gpsimd.memset(zl[:], 0.0)
                for c in range(NCHUNK):
                    nc.gpsimd.memset(vt[c][:, DH:DH + 1], 1.0)

                # QK^T: [128, 2048] = [WqT*s | WkT]^T @ X, by 512-col window
                for w in range(4):
                    ps = p_pj.tile([128, 512], f32, tag="pj")
                    for kc in range(4):
                        nc.tensor.matmul(
                            ps[:],
                            wqks[kc][:],
                            xts[kc][:, w * 512:(w + 1) * 512],
                            start=(kc == 0),
                            stop=(kc == 3),
                        )
                    nc.vector.tensor_copy(
                        qt[:, w * 512:(w + 1) * 512], ps[0:64, :]
                    )
                    nc.vector.tensor_copy(
                        kt[:, w * 512:(w + 1) * 512], ps[64:128, :]
                    )

                # V chunks in natural [k, dh] layout, bf16
                for c in range(NCHUNK):
                    pv = p_pv.tile([128, DH], f32, tag="pv")
                    for kc in range(4):
                        nc.tensor.matmul(
                            pv[:],
                            xts[kc][:, c * 128:(c + 1) * 128],
                            wvs[kc][:],
                            start=(kc == 0),
                            stop=(kc == 3),
                        )
                    nc.vector.tensor_copy(vt[c][:, 0:DH], pv[:])

            # ---------------- attention + readout ----------------
            with (
                tc.tile_pool(name="ps", bufs=2, space="PSUM") as p_s,
                tc.tile_pool(name="pacc", bufs=1, space="PSUM") as p_acc,
                tc.tile_pool(name="pop", bufs=1, space="PSUM") as p_op,
                tc.tile_pool(name="pe", bufs=3) as p_e,
                tc.tile_pool(name="pel", bufs=2) as p_el,
                tc.tile_pool(name="pcomb", bufs=2) as p_comb,
                tc.tile_pool(name="pct", bufs=2) as p_ct,
                tc.tile_pool(name="prr", bufs=3) as p_rr,
                tc.tile_pool(name="pt1", bufs=2) as p_t1,
            ):
                for hq in range(NHALF):
                    qlo = hq * QHALF
                    acc_tiles = [
                        p_acc.tile([128, 130 * len(g)], f32, tag=f"acc{t}",
                                   name=f"acc{t}_{hq}")
                        for t, g in enumerate(ACC_GROUPS)
                    ]
                    comb = p_comb.tile([128, NSUB * 128], bf16, tag="comb",
                                       name=f"comb{hq}")
                    combT = p_ct.tile([128, NSUB * 128], bf16, tag="ct",
                                      name=f"combT{hq}")
                    nc.gpsimd.memset(comb[:], 0.0)
                    # zero-init accumulators: one start=True matmul per bank
                    # (zl is zeros, so every region starts at 0 with
                    # has_written set -> all real AV matmuls are start=False
                    # accumulations in any order)
                    for t, g in enumerate(ACC_GROUPS):
                        w = 130 * len(g)
                        nc.tensor.matmul(
                            acc_tiles[t][:, 0:w],
                            zl[:, 0:128],
                            zl[:, 0:w],
                            start=True,
                            stop=False,
                            skip_group_check=True,
                        )
                    for c in range(NCHUNK):
                        sc = p_s.tile([128, QHALF], f32, tag="s")
                        for n in range(QHALF // 512):
                            nc.tensor.matmul(
                                sc[:, n * 512:(n + 1) * 512],
                                kt[:, c * 128:(c + 1) * 128],
                                qt[:, qlo + n * 512:qlo + (n + 1) * 512],
                                start=True,
                                stop=True,
                            )
                        ec = p_e.tile([128, QHALF], bf16, tag="e")
                        nc.scalar.activation(ec[:], sc[:], AF.Exp)
                        # global AV per query subtile
                        for sl in range(NSUB):
                            at, off = acc_region(acc_tiles, sl)
                            nc.tensor.matmul(
                                at[:, off:off + 65],
                                ec[:, sl * 128:(sl + 1) * 128],
                                vt[c][:],
                                start=False,
                                stop=(c == NCHUNK - 1),
                                skip_group_check=True,
                            )
                        # local AV: mask the aligned band, one matmul per
                        # overlapped subtile
                        if (hq, c) in lav:
                            s_lo, s_hi, subs = lav[(hq, c)]
                            alo, _ = aband[c]
                            w = s_hi - s_lo
                            el = p_el.tile([128, MB], bf16, tag="el")
                            nc.vector.tensor_mul(
                                el[:, 0:w],
                                ec[:, s_lo - qlo:s_hi - qlo],
                                maskt[:, c * MB + (s_lo - alo):
                                      c * MB + (s_hi - alo)],
                            )
                            for sg in subs:
                                sl = sg - hq * NSUB
                                at, off = acc_region(acc_tiles, sl)
                                nc.tensor.matmul(
                                    at[:, off + 65:off + 130],
                                    el[:, sg * 128 - s_lo:sg * 128 - s_lo + 128],
                                    vt[c][:],
                                    start=False,
                                    stop=(c == sub_chunks[sg][-1]),
                                    skip_group_check=True,
                                )
                    # -------- readout: rescale+combine -> project -> out
                    rrs = []
                    for t, g in enumerate(ACC_GROUPS):
                        w = 130 * len(g)
                        rr = p_rr.tile([128, 6], f32, tag="rr", name=f"rr{hq}_{t}")
                        nc.vector.reciprocal(
                            rr[:, 0:2 * len(g)], acc_tiles[t][:, 64:w:65]
                        )
                        nc.vector.tensor_mul(
                            rr[:, 0:2 * len(g)], rr[:, 0:2 * len(g)],
                            smalls[:, 0:2 * len(g)]
                        )
                        rrs.append(rr)
                    for sl in range(NSUB):
                        sg = hq * NSUB + sl
                        at, off = acc_region(acc_tiles, sl)
                        rr = rrs[0] if sl < 3 else (rrs[1] if sl < 6 else rrs[2])
                        kk = (sl % 3) if sl < 6 else (sl - 6)
                        t1 = p_t1.tile([128, DH], f32, tag="t1")
                        # t1 = G * ((1-g)/l_g)
                        nc.vector.tensor_scalar_mul(
                            t1[:], at[:, off:off + DH], rr[:, 2 * kk:2 * kk + 1]
                        )
                        # comb = L * (g/l_l) + t1
                        nc.vector.scalar_tensor_tensor(
                            comb[:, sl * 128:sl * 128 + DH],
                            at[:, off + 65:off + 65 + DH],
                            rr[:, 2 * kk + 1:2 * kk + 2],
                            t1[:],
                            OP.mult,
                            OP.add,
                        )
                        # transpose comb subtile via DMA XBAR (sbuf->sbuf)
                        nc.sync.dma_start_transpose(
                            combT[:, sl * 128:(sl + 1) * 128],
                            comb[:, sl * 128:(sl + 1) * 128],
                        )
                        op = p_op.tile([128, D], f32, tag="op")
                        nc.tensor.matmul(
                            op[:],
                            combT[0:DH, sl * 128:(sl + 1) * 128],
                            wot[:],
                            start=True,
                            stop=True,
                        )
                        # psum -> sbuf: DVE normally; Act for half 1 evens
                        # (Act is idle once the last exp has issued)
                        if hq == 1 and sl % 2 == 0:
                            nc.scalar.copy(outbuf[:, sg * D:(sg + 1) * D], op[:])
                        else:
                            nc.vector.tensor_copy(
                                outbuf[:, sg * D:(sg + 1) * D], op[:]
                            )
                        if sl % 2 == 1:
                            g0 = sg - 1
                            dst = out_d[g0 * 128:(g0 + 2) * 128, :].rearrange(
                                "(j p) c -> p j c", p=128
                            )
                            src = outbuf[:, g0 * D:(g0 + 2) * D].rearrange(
                                "p (j c) -> p j c", j=2
                            )
                            nc.sync.dma_start(dst, src)

    _legalize_waits(nc, mybir)
    return nc


_CACHE = {}


def _get_built(bar_key, bars):
    if bar_key not in _CACHE:
        _CACHE[bar_key] = _build(bars)
    return _CACHE[bar_key]


def _np_reference(hidden_states, bar_positions, attention_mask, Wq, bq, Wk, bk,
                  Wv, bv, Wo, bo, bar_emb, gate):
    """Plain numpy fallback (only used if inputs violate baked assumptions)."""
    B, S_, _ = hidden_states.shape
    x = hidden_states.astype(np.float64)
    q = (x @ Wq.T + bq).reshape(B, S_, H, DH).transpose(0, 2, 1, 3)
    k = (x @ Wk.T + bk).reshape(B, S_, H, DH).transpose(0, 2, 1, 3)
    v = (x @ Wv.T + bv).reshape(B, S_, H, DH).transpose(0, 2, 1, 3)
    scores = np.einsum("bhqd,bhkd->bhqk", q, k) * SCALE
    pad = attention_mask[:, None, None, :]
    bar_mask = (bar_positions[:, :, None] == bar_positions[:, None, :])[:, None]
    NEG = -np.inf

    def softmax(s):
        s = s - s.max(-1, keepdims=True)
        e = np.exp(s)
        return e / e.sum(-1, keepdims=True)

    local = softmax(np.where(bar_mask & pad, scores, NEG))
    emb = bar_emb[np.asarray(bar_positions) % bar_emb.shape[0]]
    bias = np.sum(emb * emb, axis=-1)
    glob = softmax(np.where(pad, scores + bias[:, None, :, None], NEG))
    la = np.einsum("bhqk,bhkd->bhqd", local, v)
    ga = np.einsum("bhqk,bhkd->bhqd", glob, v)
    g = 1.0 / (1.0 + np.exp(-gate))[None, :, None, None]
    comb = g * la + (1.0 - g) * ga
    out = comb.transpose(0, 2, 1, 3).reshape(B, S_, H * DH)
    return (out @ Wo.T + bo).astype(np.float32)


def kernel(**inputs):
    import ml_dtypes

    bf = ml_dtypes.bfloat16
    hidden_states = np.asarray(inputs["hidden_states"], dtype=np.float32)
    bar_positions = np.asarray(inputs["bar_positions"])
    attention_mask = np.asarray(inputs["attention_mask"])
    Wq = np.asarray(inputs["Wq"], dtype=np.float32)
    bq = np.asarray(inputs["bq"], dtype=np.float32)
    Wk = np.asarray(inputs["Wk"], dtype=np.float32)
    bk = np.asarray(inputs["bk"], dtype=np.float32)
    Wv = np.asarray(inputs["Wv"], dtype=np.float32)
    bv = np.asarray(inputs["bv"], dtype=np.float32)
    Wo = np.asarray(inputs["Wo"], dtype=np.float32)
    bo = np.asarray(inputs["bo"], dtype=np.float32)
    gate = np.asarray(inputs["gate"], dtype=np.float32)

    bp = bar_positions[0].astype(np.int64)
    if (
        hidden_states.shape != (1, S, D)
        or not bool(attention_mask.all())
        or not bool((np.diff(bp) >= 0).all())
        or np.any(bq)
        or np.any(bk)
    ):
        return _np_reference(
            hidden_states, bar_positions, attention_mask, Wq, bq, Wk, bk,
            Wv, bv, Wo, bo, np.asarray(inputs["bar_emb"], dtype=np.float32), gate,
        )

    bars = _bar_bounds(bp)
    nc = _get_built(bp.tobytes(), bars)

    # aligned mask bands (same for every core)
    aband = _abands(bars)
    maskband = np.zeros((128, NCHUNK * MB), dtype=bf)
    for c in range(NCHUNK):
        alo, ahi = aband[c]
        eq = (bp[c * 128:(c + 1) * 128, None] == bp[None, alo:ahi])
        maskband[:, c * MB:c * MB + (ahi - alo)] = eq.astype(bf)

    xt = np.ascontiguousarray(hidden_states[0].T).astype(bf)  # [512, 2048]
    g = 1.0 / (1.0 + np.exp(-gate.astype(np.float64)))  # sigmoid, [H]
    in_maps = []
    for h in range(H):
        sl = slice(h * DH, (h + 1) * DH)
        wqk = np.empty((D, 128), dtype=np.float32)
        wqk[:, 0:64] = Wq[sl, :].T * np.float32(SCALE)
        wqk[:, 64:128] = Wk[sl, :].T
        smalls = np.zeros((128, 8), dtype=np.float32)
        smalls[:, 0:6:2] = np.float32(1.0 - g[h])
        smalls[:, 1:6:2] = np.float32(g[h])
        in_maps.append({
            "xt": xt,
            "wqk": wqk.astype(bf),
            "wv": np.ascontiguousarray(Wv[sl, :].T).astype(bf),
            "wot": np.ascontiguousarray(Wo[:, sl].T).astype(bf),
            "smalls": smalls,
            "maskband": maskband,
        })

    res = _run_spmd(nc, in_maps)
    out = np.zeros((S, D), dtype=np.float32)
    for h in range(H):
        out += res.results[h]["out_partial"]
    out += bo + Wo @ bv
    return out.reshape(1, S, D)


def _run_spmd(nc, in_maps, **kw):
    from concourse.bass_utils import run_bass_kernel_spmd

    return run_bass_kernel_spmd(nc, in_maps, list(range(H)), **kw)


# revision 7
# speedup vs baseline: 1.2873x; 1.2157x over previous
"""Bar-level attention Trainium2 kernel (8 NeuronCores, head-parallel).

Contract: kernel(**inputs) takes the FULL inputs from setup_inputs() and
returns the FULL [1, 2048, 512] float32 output.

Strategy (one head per core, 8 heads / 8 cores), v2 layout:
  - Host: XT [512, 2048] bf16; one packed weight wall per head
    ([WqT*scale | WkT] | WvT | WoT) bf16; sigmoid(gate) folded into
    per-partition scalars; bq/bk are zero in setup_inputs (guarded);
    bv/bo folded into the host-side bias add (exact: softmax rows sum
    to 1, so the V bias passes straight through both branches).
  - Device (per core):
      Q^T/K^T [64, 2048] f32r (joint [128, *] projection, split on copy),
      V [k, 65] bf16 per 128-key chunk (col 64 = ones -> denominators).
      Attention per 1024-query half, per 128-key chunk:
        S^T = K_chunk @ Q^T -> psum [128, 1024] (keys on partitions)
        E = exp(S^T) -> bf16 (no max subtraction: scores ~ N(0,1))
        global AV: acc[q_sub 128, 65] += E_sub^T @ V (8 query subtiles,
          full 128-partition contraction; G/L accumulators packed
          3-subtiles-per-psum-bank, zero-initialized by one start=True
          matmul against a zeroed tile so every real AV matmul is an
          order-independent start=False accumulation)
        local AV: same, with E masked to same-bar keys (host-shipped
          128-aligned mask band per chunk).
      Readout per half: strided reciprocal over the packed denominator
      columns, gate fold, rescale+combine -> comb [q, 64] bf16; batched
      DMA-XBAR transpose -> comb^T; output projection [q 128, 512] per
      subtile; psum->sbuf copy; batched DMA out. Half-0's projection is
      spread across half-1's chunk loop to keep PE/Act dense.
  - Host: sum the 8 partial outputs + bo + Wo@bv.

DMA count is minimized (each dma_start costs ~625ns of serialized HWDGE
dispatch): 12 input DMAs, 6 XBAR transposes, 6 output DMAs.
"""

import numpy as np

S = 2048
D = 512
H = 8
DH = 64
SCALE = 1.0 / np.sqrt(DH)
NCHUNK = S // 128       # 16 key chunks of 128
NHALF = 2               # query halves of 1024
QHALF = S // NHALF
NSUB = QHALF // 128     # 8 query subtiles per half
MB = 640                # aligned mask band width per chunk
ACC_GROUPS = [(0, 1, 2), (3, 4, 5), (6, 7)]
WALL_W = 1280           # wqk 512 | wv 256 | wot 512


def _legalize_waits(nc, mybir):
    """This walrus codegen accepts at most ONE sync wait per instruction.
    Split any instruction carrying N>1 waits into N-1 preceding single-wait
    NoOps on the same engine (waits execute in order on the sequencer)."""
    ctr = 0
    for f in nc.m.functions:
        for b in f.blocks:
            insts = b.instructions
            if not any(i.sync_info and len(i.sync_info.on_wait) > 1 for i in insts):
                continue
            new = []
            for ins in insts:
                si = ins.sync_info
                if si is not None and len(si.on_wait) > 1:
                    waits = list(si.on_wait)
                    for w in waits[:-1]:
                        ctr += 1
                        nop = mybir.InstNoOp(name=f"waitsplit-{ctr}", engine=ins.engine)
                        nop.sync_info = mybir.SyncInfo(on_wait=[w], on_update=[])
                        new.append(nop)
                    ins.sync_info = mybir.SyncInfo(
                        on_wait=[waits[-1]], on_update=list(si.on_update)
                    )
                new.append(ins)
            insts.clear()
            insts.extend(new)
    return ctr


def _bar_bounds(bp):
    """bp: sorted int array [S] -> list of (start, end) per bar."""
    change = np.nonzero(np.diff(bp))[0] + 1
    starts = np.concatenate([[0], change])
    ends = np.concatenate([change, [len(bp)]])
    return list(zip(starts.tolist(), ends.tolist()))


def _abands(bars):
    """Per chunk: 128-aligned query band of bars intersecting the chunk."""
    ab = []
    for c in range(NCHUNK):
        klo, khi = c * 128, (c + 1) * 128
        bs = [b for b in bars if b[1] > klo and b[0] < khi]
        blo, bhi = bs[0][0], bs[-1][1]
        alo = (blo // 128) * 128
        ahi = ((bhi + 127) // 128) * 128
        assert ahi - alo <= MB
        ab.append((alo, ahi))
    return ab


def _build(bars):
    import concourse.bass as bass
    import concourse.tile as tile
    import concourse.mybir as mybir

    dt = mybir.dt
    AF = mybir.ActivationFunctionType
    OP = mybir.AluOpType
    f32 = dt.float32
    f32r = dt.float32r
    bf16 = dt.bfloat16

    nc = bass.Bass()
    xt_d = nc.dram_tensor("xt", [D, S], bf16, kind="ExternalInput")
    wall_d = nc.dram_tensor("wall", [128, WALL_W], bf16, kind="ExternalInput")
    smalls_d = nc.dram_tensor("smalls", [128, 8], f32, kind="ExternalInput")
    mask_d = nc.dram_tensor("maskband", [128, NCHUNK * MB], bf16, kind="ExternalInput")
    out_d = nc.dram_tensor("out_partial", [S, D], f32, kind="ExternalOutput")

    aband = _abands(bars)
    lav = {}  # (hq, c) -> (s_lo, s_hi, [global subtile indices])
    sub_chunks = {}  # global subtile -> chunks writing its L region
    for hq in range(NHALF):
        qlo, qhi = hq * QHALF, (hq + 1) * QHALF
        for c in range(NCHUNK):
            alo, ahi = aband[c]
            s_lo, s_hi = max(alo, qlo), min(ahi, qhi)
            if s_lo >= s_hi:
                continue
            subs = list(range(s_lo // 128, s_hi // 128))
            lav[(hq, c)] = (s_lo, s_hi, subs)
            for sg in subs:
                sub_chunks.setdefault(sg, []).append(c)
    assert all(sg in sub_chunks for sg in range(NHALF * NSUB))

    def acc_region(acc_tiles, s_loc):
        for t, grp in enumerate(ACC_GROUPS):
            if s_loc in grp:
                return acc_tiles[t], grp.index(s_loc) * 130, t
        raise AssertionError

    with tile.TileContext(nc, pool_alloc_mode="queue") as tc:
        with (
            tc.tile_pool(name="persist", bufs=1) as p_keep,
            tc.tile_pool(name="outb", bufs=1) as p_out,
        ):
            qt = p_keep.tile([DH, S], f32r, tag="qt")
            kt = p_keep.tile([DH, S], f32r, tag="kt")
            vt = [p_keep.tile([128, DH + 1], bf16, tag=f"vt{c}", name=f"vt{c}")
                  for c in range(NCHUNK)]
            maskt = p_keep.tile([128, NCHUNK * MB], bf16, tag="maskt")
            smalls = p_keep.tile([128, 8], f32, tag="smalls")
            wall = p_keep.tile([128, WALL_W], bf16, tag="wall")
            zl = p_keep.tile([128, 390], bf16, tag="zl")
            outbuf = p_out.tile([128, NHALF * NSUB * D], f32, tag="outbuf")
            wqks = [wall[:, kc * 128:(kc + 1) * 128] for kc in range(4)]
            wvs = [wall[:, 512 + kc * DH:512 + (kc + 1) * DH] for kc in range(4)]
            wot = wall[0:DH, 768:768 + D]

            # ---------------- input DMAs ----------------
            nc.sync.dma_start(wall[:], wall_d[:])
            nc.sync.dma_start(smalls[:], smalls_d[:])
            with (
                tc.tile_pool(name="inp", bufs=1) as p_in,
                tc.tile_pool(name="pj", bufs=2, space="PSUM") as p_pj,
                tc.tile_pool(name="pv", bufs=2, space="PSUM") as p_pv,
            ):
                xts = [p_in.tile([128, S], bf16, tag=f"xt{i}", name=f"xts{i}")
                       for i in range(4)]
                for p in range(2):
                    for kc in range(4):
                        nc.sync.dma_start(
                            xts[kc][:, p * 1024:(p + 1) * 1024],
                            xt_d[kc * 128:(kc + 1) * 128, p * 1024:(p + 1) * 1024],
                        )
                nc.sync.dma_start(
                    maskt[:, 0:8 * MB], mask_d[:, 0:8 * MB]
                )
                nc.sync.dma_start(
                    maskt[:, 8 * MB:NCHUNK * MB], mask_d[:, 8 * MB:NCHUNK * MB]
                )
                # zeros + ones-column init (Pool engine, no deps)
                nc.gpsimd.memset(zl[:], 0.0)
                for c in range(NCHUNK):
                    nc.gpsimd.memset(vt[c][:, DH:DH + 1], 1.0)

                # ---------------- projections ----------------
                def qk_window(w):
                    ps = p_pj.tile([128, 512], f32, tag="pj")
                    for kc in range(4):
                        nc.tensor.matmul(
                            ps[:],
                            wqks[kc],
                            xts[kc][:, w * 512:(w + 1) * 512],
                            start=(kc == 0),
                            stop=(kc == 3),
                        )
                    nc.vector.tensor_copy(qt[:, w * 512:(w + 1) * 512], ps[0:64, :])
                    nc.vector.tensor_copy(kt[:, w * 512:(w + 1) * 512], ps[64:128, :])

                def v_chunk(c):
                    pv = p_pv.tile([128, DH], f32, tag="pv")
                    for kc in range(4):
                        nc.tensor.matmul(
                            pv[:],
                            xts[kc][:, c * 128:(c + 1) * 128],
                            wvs[kc],
                            start=(kc == 0),
                            stop=(kc == 3),
                        )
                    nc.vector.tensor_copy(vt[c][:, 0:DH], pv[:])

                qk_window(0)
                qk_window(1)
                for c in range(8):
                    v_chunk(c)
                qk_window(2)
                qk_window(3)
                for c in range(8, NCHUNK):
                    v_chunk(c)

            # ---------------- attention + readout ----------------
            with (
                tc.tile_pool(name="ps", bufs=2, space="PSUM") as p_s,
                tc.tile_pool(name="pacc", bufs=1, space="PSUM") as p_acc,
                tc.tile_pool(name="pop", bufs=1, space="PSUM") as p_op,
                tc.tile_pool(name="pe", bufs=4) as p_e,
                tc.tile_pool(name="pel", bufs=2) as p_el,
                tc.tile_pool(name="pcomb", bufs=2) as p_comb,
                tc.tile_pool(name="pct", bufs=2) as p_ct,
                tc.tile_pool(name="prr", bufs=3) as p_rr,
                tc.tile_pool(name="pt1", bufs=2) as p_t1,
            ):
                halves = []  # per half: (acc_tiles, comb, combT, rrs)

                def emit_init(hq, acc_tiles):
                    for t, g in enumerate(ACC_GROUPS):
                        w = 130 * len(g)
                        nc.tensor.matmul(
                            acc_tiles[t][:, 0:w],
                            zl[:, 0:128],
                            zl[:, 0:w],
                            start=True,
                            stop=False,
                            skip_group_check=True,
                        )

                def emit_scores_exp(hq, c):
                    qlo = hq * QHALF
                    sc = p_s.tile([128, QHALF], f32, tag="s", name=f"sc{hq}_{c}")
                    for n in range(QHALF // 512):
                        nc.tensor.matmul(
                            sc[:, n * 512:(n + 1) * 512],
                            kt[:, c * 128:(c + 1) * 128],
                            qt[:, qlo + n * 512:qlo + (n + 1) * 512],
                            start=True,
                            stop=True,
                        )
                    ec = p_e.tile([128, QHALF], bf16, tag="e", name=f"ec{hq}_{c}")
                    nc.scalar.activation(ec[:], sc[:], AF.Exp)
                    return ec

                def emit_av(hq, c, ec, acc_tiles):
                    qlo = hq * QHALF
                    for sl in range(NSUB):
                        at, off, _ = acc_region(acc_tiles, sl)
                        nc.tensor.matmul(
                            at[:, off:off + 65],
                            ec[:, sl * 128:(sl + 1) * 128],
                            vt[c][:],
                            start=False,
                            stop=(c == NCHUNK - 1),
                            skip_group_check=True,
                        )
                    if (hq, c) in lav:
                        s_lo, s_hi, subs = lav[(hq, c)]
                        alo, _ = aband[c]
                        w = s_hi - s_lo
                        el = p_el.tile([128, MB], bf16, tag="el")
                        nc.vector.tensor_mul(
                            el[:, 0:w],
                            ec[:, s_lo - qlo:s_hi - qlo],
                            maskt[:, c * MB + (s_lo - alo):c * MB + (s_hi - alo)],
                        )
                        for sg in subs:
                            sl = sg - hq * NSUB
                            at, off, _ = acc_region(acc_tiles, sl)
                            nc.tensor.matmul(
                                at[:, off + 65:off + 130],
                                el[:, sg * 128 - s_lo:sg * 128 - s_lo + 128],
                                vt[c][:],
                                start=False,
                                stop=(c == sub_chunks[sg][-1]),
                                skip_group_check=True,
                            )

                def emit_rescale(hq):
                    acc_tiles, comb, _, rrs = halves[hq]
                    for t, g in enumerate(ACC_GROUPS):
                        w = 130 * len(g)
                        rr = p_rr.tile([128, 6], f32, tag="rr", name=f"rr{hq}_{t}")
                        nc.vector.reciprocal(
                            rr[:, 0:2 * len(g)], acc_tiles[t][:, 64:w:65]
                        )
                        nc.vector.tensor_mul(
                            rr[:, 0:2 * len(g)], rr[:, 0:2 * len(g)],
                            smalls[:, 0:2 * len(g)]
                        )
                        rrs.append(rr)
                    for sl in range(NSUB):
                        at, off, t = acc_region(acc_tiles, sl)
                        rr = rrs[t]
                        kk = sl - (0, 3, 6)[t]
                        t1 = p_t1.tile([128, DH], f32, tag="t1")
                        nc.vector.tensor_scalar_mul(
                            t1[:], at[:, off:off + DH], rr[:, 2 * kk:2 * kk + 1]
                        )
                        nc.vector.scalar_tensor_tensor(
                            comb[:, sl * 128:sl * 128 + DH],
                            at[:, off + 65:off + 65 + DH],
                            rr[:, 2 * kk + 1:2 * kk + 2],
                            t1[:],
                            OP.mult,
                            OP.add,
                        )

                def emit_xbar(hq, sl0, nsl):
                    _, comb, combT, _ = halves[hq]
                    nc.sync.dma_start_transpose(
                        combT[:, sl0 * 128:(sl0 + nsl) * 128].rearrange(
                            "p (j c) -> p j c", j=nsl
                        ),
                        comb[:, sl0 * 128:(sl0 + nsl) * 128],
                    )

                def emit_proj(hq, sl, op_tile, copy_eng):
                    _, _, combT, _ = halves[hq]
                    sg = hq * NSUB + sl
                    nc.tensor.matmul(
                        op_tile[:, 0:D],
                        combT[0:DH, sl * 128:(sl + 1) * 128],
                        wot,
                        start=True,
                        stop=True,
                    )
                    if copy_eng == "act":
                        nc.scalar.copy(outbuf[:, sg * D:(sg + 1) * D], op_tile[:, 0:D])
                    else:
                        nc.vector.tensor_copy(
                            outbuf[:, sg * D:(sg + 1) * D], op_tile[:, 0:D]
                        )

                def emit_outdma(hq, sl0, nsl):
                    g0 = hq * NSUB + sl0
                    dst = out_d[g0 * 128:(g0 + nsl) * 128, :].rearrange(
                        "(j p) c -> p j c", p=128
                    )
                    src = outbuf[:, g0 * D:(g0 + nsl) * D].rearrange(
                        "p (j c) -> p j c", j=nsl
                    )
                    nc.sync.dma_start(dst, src)

                def new_half(hq):
                    acc_tiles = [
                        p_acc.tile([128, 130 * len(g)], f32, tag=f"acc{t}",
                                   name=f"acc{t}_{hq}")
                        for t, g in enumerate(ACC_GROUPS)
                    ]
                    comb = p_comb.tile([128, NSUB * 128], bf16, tag="comb",
                                       name=f"comb{hq}")
                    combT = p_ct.tile([128, NSUB * 128], bf16, tag="ct",
                                      name=f"combT{hq}")
                    nc.gpsimd.memset(comb[:], 0.0)
                    halves.append((acc_tiles, comb, combT, []))
                    return acc_tiles

                # ---- half 0 ----
                acc0 = new_half(0)
                emit_init(0, acc0)
                for c in range(NCHUNK):
                    ec = emit_scores_exp(0, c)
                    emit_av(0, c, ec, acc0)
                emit_rescale(0)

                # ---- half 1, with half-0 projection spread through it ----
                acc1 = new_half(1)
                pend = []  # deferred (hq, c, ec) AV blocks before init
                for c in range(2):
                    pend.append((c, emit_scores_exp(1, c)))
                emit_init(1, acc1)
                for c, ec in pend:
                    emit_av(1, c, ec, acc1)
                for c in range(2, NCHUNK):
                    ec = emit_scores_exp(1, c)
                    emit_av(1, c, ec, acc1)
                    # interleave half-0 projection: one subtile per chunk
                    if c == 2:
                        emit_xbar(0, 0, 4)
                    if c == 7:
                        emit_xbar(0, 4, 4)
                    if 3 <= c <= 10:
                        sl = c - 3
                        op = p_op.tile([128, D], f32, tag="op", name=f"op0_{sl}")
                        emit_proj(0, sl, op, "dve")
                        if sl % 4 == 3:
                            emit_outdma(0, sl - 3, 4)

                # ---- half-1 readout (tail) ----
                emit_rescale(1)
                for sl in range(NSUB):
                    if sl % 2 == 0:
                        emit_xbar(1, sl, 2)
                    # rotate 3 psum buffers: op pool + two score-pool tiles
                    if sl % 3 == 0:
                        op = p_op.tile([128, D], f32, tag="op", name=f"op1_{sl}")
                    else:
                        op = p_s.tile([128, QHALF], f32, tag="s", name=f"ops1_{sl}")
                    emit_proj(1, sl, op, "act" if sl % 2 == 0 else "dve")
                    if sl % 2 == 1:
                        emit_outdma(1, sl - 1, 2)

    _legalize_waits(nc, mybir)
    return nc


_CACHE = {}


def _get_built(bar_key, bars):
    if bar_key not in _CACHE:
        _CACHE[bar_key] = _build(bars)
    return _CACHE[bar_key]


def _np_reference(hidden_states, bar_positions, attention_mask, Wq, bq, Wk, bk,
                  Wv, bv, Wo, bo, bar_emb, gate):
    """Plain numpy fallback (only used if inputs violate baked assumptions)."""
    B, S_, _ = hidden_states.shape
    x = hidden_states.astype(np.float64)
    q = (x @ Wq.T + bq).reshape(B, S_, H, DH).transpose(0, 2, 1, 3)
    k = (x @ Wk.T + bk).reshape(B, S_, H, DH).transpose(0, 2, 1, 3)
    v = (x @ Wv.T + bv).reshape(B, S_, H, DH).transpose(0, 2, 1, 3)
    scores = np.einsum("bhqd,bhkd->bhqk", q, k) * SCALE
    pad = attention_mask[:, None, None, :]
    bar_mask = (bar_positions[:, :, None] == bar_positions[:, None, :])[:, None]
    NEG = -np.inf

    def softmax(s):
        s = s - s.max(-1, keepdims=True)
        e = np.exp(s)
        return e / e.sum(-1, keepdims=True)

    local = softmax(np.where(bar_mask & pad, scores, NEG))
    emb = bar_emb[np.asarray(bar_positions) % bar_emb.shape[0]]
    bias = np.sum(emb * emb, axis=-1)
    glob = softmax(np.where(pad, scores + bias[:, None, :, None], NEG))
    la = np.einsum("bhqk,bhkd->bhqd", local, v)
    ga = np.einsum("bhqk,bhkd->bhqd", glob, v)
    g = 1.0 / (1.0 + np.exp(-gate))[None, :, None, None]
    comb = g * la + (1.0 - g) * ga
    out = comb.transpose(0, 2, 1, 3).reshape(B, S_, H * DH)
    return (out @ Wo.T + bo).astype(np.float32)


def kernel(**inputs):
    import ml_dtypes

    bf = ml_dtypes.bfloat16
    hidden_states = np.asarray(inputs["hidden_states"], dtype=np.float32)
    bar_positions = np.asarray(inputs["bar_positions"])
    attention_mask = np.asarray(inputs["attention_mask"])
    Wq = np.asarray(inputs["Wq"], dtype=np.float32)
    bq = np.asarray(inputs["bq"], dtype=np.float32)
    Wk = np.asarray(inputs["Wk"], dtype=np.float32)
    bk = np.asarray(inputs["bk"], dtype=np.float32)
    Wv = np.asarray(inputs["Wv"], dtype=np.float32)
    bv = np.asarray(inputs["bv"], dtype=np.float32)
    Wo = np.asarray(inputs["Wo"], dtype=np.float32)
    bo = np.asarray(inputs["bo"], dtype=np.float32)
    gate = np.asarray(inputs["gate"], dtype=np.float32)

    bp = bar_positions[0].astype(np.int64)
    if (
        hidden_states.shape != (1, S, D)
        or not bool(attention_mask.all())
        or not bool((np.diff(bp) >= 0).all())
        or np.any(bq)
        or np.any(bk)
    ):
        return _np_reference(
            hidden_states, bar_positions, attention_mask, Wq, bq, Wk, bk,
            Wv, bv, Wo, bo, np.asarray(inputs["bar_emb"], dtype=np.float32), gate,
        )

    bars = _bar_bounds(bp)
    nc = _get_built(bp.tobytes(), bars)

    # aligned mask bands (same for every core)
    aband = _abands(bars)
    maskband = np.zeros((128, NCHUNK * MB), dtype=bf)
    for c in range(NCHUNK):
        alo, ahi = aband[c]
        eq = (bp[c * 128:(c + 1) * 128, None] == bp[None, alo:ahi])
        maskband[:, c * MB:c * MB + (ahi - alo)] = eq.astype(bf)

    xt = np.ascontiguousarray(hidden_states[0].T).astype(bf)  # [512, 2048]
    g = 1.0 / (1.0 + np.exp(-gate.astype(np.float64)))  # sigmoid, [H]
    in_maps = []
    for h in range(H):
        sl = slice(h * DH, (h + 1) * DH)
        wall = np.zeros((128, WALL_W), dtype=np.float32)
        for kc in range(4):
            r = slice(kc * 128, (kc + 1) * 128)
            wall[:, kc * 128:kc * 128 + 64] = Wq[sl, r].T * np.float32(SCALE)
            wall[:, kc * 128 + 64:(kc + 1) * 128] = Wk[sl, r].T
            wall[:, 512 + kc * DH:512 + (kc + 1) * DH] = Wv[sl, r].T
        wall[0:DH, 768:768 + D] = Wo[:, sl].T
        smalls = np.zeros((128, 8), dtype=np.float32)
        smalls[:, 0:6:2] = np.float32(1.0 - g[h])
        smalls[:, 1:6:2] = np.float32(g[h])
        in_maps.append({
            "xt": xt,
            "wall": wall.astype(bf),
            "smalls": smalls,
            "maskband": maskband,
        })

    res = _run_spmd(nc, in_maps)
    out = np.zeros((S, D), dtype=np.float32)
    for h in range(H):
        out += res.results[h]["out_partial"]
    out += bo + Wo @ bv
    return out.reshape(1, S, D)


def _run_spmd(nc, in_maps, **kw):
    from concourse.bass_utils import run_bass_kernel_spmd

    return run_bass_kernel_spmd(nc, in_maps, list(range(H)), **kw)


# revision 10
# speedup vs baseline: 1.3611x; 1.0573x over previous
"""Bar-level attention Trainium2 kernel (8 NeuronCores, head-parallel).

Contract: kernel(**inputs) takes the FULL inputs from setup_inputs() and
returns the FULL [1, 2048, 512] float32 output.

Strategy (one head per core, 8 heads / 8 cores), v2 layout:
  - Host: XT [512, 2048] bf16; one packed weight wall per head
    ([WqT*scale | WkT] | WvT | WoT) bf16; sigmoid(gate) folded into
    per-partition scalars; bq/bk are zero in setup_inputs (guarded);
    bv/bo folded into the host-side bias add (exact: softmax rows sum
    to 1, so the V bias passes straight through both branches).
  - Device (per core):
      Q^T/K^T [64, 2048] f32r (joint [128, *] projection, split on copy),
      V [k, 65] bf16 per 128-key chunk (col 64 = ones -> denominators).
      Attention per 1024-query half, per 128-key chunk:
        S^T = K_chunk @ Q^T -> psum [128, 1024] (keys on partitions)
        E = exp(S^T) -> bf16 (no max subtraction: scores ~ N(0,1))
        global AV: acc[q_sub 128, 65] += E_sub^T @ V (8 query subtiles,
          full 128-partition contraction; G/L accumulators packed
          3-subtiles-per-psum-bank, zero-initialized by one start=True
          matmul against a zeroed tile so every real AV matmul is an
          order-independent start=False accumulation)
        local AV: same, with E masked to same-bar keys (host-shipped
          128-aligned mask band per chunk).
      Readout per half: strided reciprocal over the packed denominator
      columns, gate fold, rescale+combine -> comb [q, 64] bf16; batched
      DMA-XBAR transpose -> comb^T; output projection [q 128, 512] per
      subtile; psum->sbuf copy; batched DMA out. Half-0's projection is
      spread across half-1's chunk loop to keep PE/Act dense.
  - Host: sum the 8 partial outputs + bo + Wo@bv.

DMA count is minimized (each dma_start costs ~625ns of serialized HWDGE
dispatch): 12 input DMAs, 6 XBAR transposes, 6 output DMAs.
"""

import numpy as np

S = 2048
D = 512
H = 8
DH = 64
SCALE = 1.0 / np.sqrt(DH)
NCHUNK = S // 128       # 16 key chunks of 128
NHALF = 2               # query halves of 1024
QHALF = S // NHALF
NSUB = QHALF // 128     # 8 query subtiles per half
MB = 640                # aligned mask band width per chunk
ACC_GROUPS = [(0, 1, 2), (3, 4, 5), (6, 7)]
WALL_W = 1280           # wqk 512 | wv 256 | wot 512


def _legalize_waits(nc, mybir):
    """This walrus codegen accepts at most ONE sync wait per instruction.
    Split any instruction carrying N>1 waits into N-1 preceding single-wait
    NoOps on the same engine (waits execute in order on the sequencer)."""
    ctr = 0
    for f in nc.m.functions:
        for b in f.blocks:
            insts = b.instructions
            if not any(i.sync_info and len(i.sync_info.on_wait) > 1 for i in insts):
                continue
            new = []
            for ins in insts:
                si = ins.sync_info
                if si is not None and len(si.on_wait) > 1:
                    waits = list(si.on_wait)
                    for w in waits[:-1]:
                        ctr += 1
                        nop = mybir.InstNoOp(name=f"waitsplit-{ctr}", engine=ins.engine)
                        nop.sync_info = mybir.SyncInfo(on_wait=[w], on_update=[])
                        new.append(nop)
                    ins.sync_info = mybir.SyncInfo(
                        on_wait=[waits[-1]], on_update=list(si.on_update)
                    )
                new.append(ins)
            insts.clear()
            insts.extend(new)
    return ctr


def _bar_bounds(bp):
    """bp: sorted int array [S] -> list of (start, end) per bar."""
    change = np.nonzero(np.diff(bp))[0] + 1
    starts = np.concatenate([[0], change])
    ends = np.concatenate([change, [len(bp)]])
    return list(zip(starts.tolist(), ends.tolist()))


def _abands(bars):
    """Per chunk: 128-aligned query band of bars intersecting the chunk."""
    ab = []
    for c in range(NCHUNK):
        klo, khi = c * 128, (c + 1) * 128
        bs = [b for b in bars if b[1] > klo and b[0] < khi]
        blo, bhi = bs[0][0], bs[-1][1]
        alo = (blo // 128) * 128
        ahi = ((bhi + 127) // 128) * 128
        assert ahi - alo <= MB
        ab.append((alo, ahi))
    return ab


def _build(bars):
    import concourse.bass as bass
    import concourse.tile as tile
    import concourse.mybir as mybir

    dt = mybir.dt
    AF = mybir.ActivationFunctionType
    OP = mybir.AluOpType
    f32 = dt.float32
    f32r = dt.float32r
    bf16 = dt.bfloat16

    nc = bass.Bass()
    xt_d = nc.dram_tensor("xt", [D, S], bf16, kind="ExternalInput")
    wall_d = nc.dram_tensor("wall", [128, WALL_W], bf16, kind="ExternalInput")
    smalls_d = nc.dram_tensor("smalls", [128, 8], f32, kind="ExternalInput")
    mask_d = nc.dram_tensor("maskband", [128, NCHUNK * MB], bf16, kind="ExternalInput")
    out_d = nc.dram_tensor("out_partial", [S, D], f32, kind="ExternalOutput")

    aband = _abands(bars)
    lav = {}  # (hq, c) -> (s_lo, s_hi, [global subtile indices])
    sub_chunks = {}  # global subtile -> chunks writing its L region
    for hq in range(NHALF):
        qlo, qhi = hq * QHALF, (hq + 1) * QHALF
        for c in range(NCHUNK):
            alo, ahi = aband[c]
            s_lo, s_hi = max(alo, qlo), min(ahi, qhi)
            if s_lo >= s_hi:
                continue
            subs = list(range(s_lo // 128, s_hi // 128))
            lav[(hq, c)] = (s_lo, s_hi, subs)
            for sg in subs:
                sub_chunks.setdefault(sg, []).append(c)
    assert all(sg in sub_chunks for sg in range(NHALF * NSUB))

    def acc_region(acc_tiles, s_loc):
        for t, grp in enumerate(ACC_GROUPS):
            if s_loc in grp:
                return acc_tiles[t], grp.index(s_loc) * 130, t
        raise AssertionError

    with tile.TileContext(nc, pool_alloc_mode="queue") as tc:
        with (
            tc.tile_pool(name="persist", bufs=1) as p_keep,
            tc.tile_pool(name="outb", bufs=1) as p_out,
        ):
            qt = p_keep.tile([DH, S], f32r, tag="qt")
            kt = p_keep.tile([DH, S], f32r, tag="kt")
            vt = [p_keep.tile([128, DH + 1], bf16, tag=f"vt{c}", name=f"vt{c}")
                  for c in range(NCHUNK)]
            maskt = p_keep.tile([128, NCHUNK * MB], bf16, tag="maskt")
            smalls = p_keep.tile([128, 8], f32, tag="smalls")
            wall = p_keep.tile([128, WALL_W], bf16, tag="wall")
            zl = p_keep.tile([128, 390], bf16, tag="zl")
            outbuf = p_out.tile([128, NHALF * NSUB * D], f32, tag="outbuf")
            wqks = [wall[:, kc * 128:(kc + 1) * 128] for kc in range(4)]
            wvs = [wall[:, 512 + kc * DH:512 + (kc + 1) * DH] for kc in range(4)]
            wot = wall[0:DH, 768:768 + D]

            # ---------------- input DMAs ----------------
            nc.sync.dma_start(wall[:], wall_d[:])
            with (
                tc.tile_pool(name="inp", bufs=1) as p_in,
                tc.tile_pool(name="ps", bufs=2, space="PSUM") as p_s,
                tc.tile_pool(name="pacc", bufs=1, space="PSUM") as p_acc,
                tc.tile_pool(name="pop", bufs=1, space="PSUM") as p_op,
                tc.tile_pool(name="pe", bufs=4) as p_e,
                tc.tile_pool(name="pel", bufs=2) as p_el,
                tc.tile_pool(name="pcomb", bufs=2) as p_comb,
                tc.tile_pool(name="pct", bufs=2) as p_ct,
                tc.tile_pool(name="prr", bufs=3) as p_rr,
                tc.tile_pool(name="pt1", bufs=2) as p_t1,
            ):
                xts = [p_in.tile([128, S], bf16, tag=f"xt{i}", name=f"xts{i}")
                       for i in range(4)]
                for p in range(2):
                    for kc in range(4):
                        nc.sync.dma_start(
                            xts[kc][:, p * 1024:(p + 1) * 1024],
                            xt_d[kc * 128:(kc + 1) * 128, p * 1024:(p + 1) * 1024],
                        )
                nc.sync.dma_start(
                    maskt[:, 0:4 * MB], mask_d[:, 0:4 * MB]
                )
                nc.sync.dma_start(
                    maskt[:, 4 * MB:NCHUNK * MB], mask_d[:, 4 * MB:NCHUNK * MB]
                )
                nc.sync.dma_start(smalls[:], smalls_d[:])
                # zeros + ones-column init (Pool engine, no deps)
                nc.gpsimd.memset(zl[:], 0.0)
                for c in range(NCHUNK):
                    nc.gpsimd.memset(vt[c][:, DH:DH + 1], 1.0)

                # ---------------- projections (psum shared with attention:
                # QK windows ride the scores pool, V chunks the op pool) ----
                def qk_window(w):
                    ps = p_s.tile([128, QHALF], f32, tag="s", name=f"qkps{w}")
                    for kc in range(4):
                        nc.tensor.matmul(
                            ps[:, 0:512],
                            wqks[kc],
                            xts[kc][:, w * 512:(w + 1) * 512],
                            start=(kc == 0),
                            stop=(kc == 3),
                        )
                    # w0/w1 q-copies ride the still-idle Act engine
                    if w < 2:
                        nc.scalar.copy(qt[:, w * 512:(w + 1) * 512], ps[0:64, 0:512])
                    else:
                        nc.vector.tensor_copy(
                            qt[:, w * 512:(w + 1) * 512], ps[0:64, 0:512]
                        )
                    nc.vector.tensor_copy(
                        kt[:, w * 512:(w + 1) * 512], ps[64:128, 0:512]
                    )

                def v_chunk(c):
                    pv = p_op.tile([128, D], f32, tag="op", name=f"vps{c}")
                    for kc in range(4):
                        nc.tensor.matmul(
                            pv[:, 0:DH],
                            xts[kc][:, c * 128:(c + 1) * 128],
                            wvs[kc],
                            start=(kc == 0),
                            stop=(kc == 3),
                        )
                    nc.vector.tensor_copy(vt[c][:, 0:DH], pv[:, 0:DH])
                halves = []  # per half: (acc_tiles, comb, combT, rrs)

                def emit_init(hq, acc_tiles):
                    for t, g in enumerate(ACC_GROUPS):
                        w = 130 * len(g)
                        nc.tensor.matmul(
                            acc_tiles[t][:, 0:w],
                            zl[:, 0:128],
                            zl[:, 0:w],
                            start=True,
                            stop=False,
                            skip_group_check=True,
                        )

                def emit_scores_exp(hq, c):
                    qlo = hq * QHALF
                    sc = p_s.tile([128, QHALF], f32, tag="s", name=f"sc{hq}_{c}")
                    for n in range(QHALF // 512):
                        nc.tensor.matmul(
                            sc[:, n * 512:(n + 1) * 512],
                            kt[:, c * 128:(c + 1) * 128],
                            qt[:, qlo + n * 512:qlo + (n + 1) * 512],
                            start=True,
                            stop=True,
                        )
                    ec = p_e.tile([128, QHALF], bf16, tag="e", name=f"ec{hq}_{c}")
                    nc.scalar.activation(ec[:], sc[:], AF.Exp)
                    return ec

                def emit_av(hq, c, ec, acc_tiles):
                    qlo = hq * QHALF
                    for sl in range(NSUB):
                        at, off, _ = acc_region(acc_tiles, sl)
                        nc.tensor.matmul(
                            at[:, off:off + 65],
                            ec[:, sl * 128:(sl + 1) * 128],
                            vt[c][:],
                            start=False,
                            stop=(c == NCHUNK - 1),
                            skip_group_check=True,
                        )
                    if (hq, c) in lav:
                        s_lo, s_hi, subs = lav[(hq, c)]
                        alo, _ = aband[c]
                        w = s_hi - s_lo
                        el = p_el.tile([128, MB], bf16, tag="el")
                        nc.vector.tensor_mul(
                            el[:, 0:w],
                            ec[:, s_lo - qlo:s_hi - qlo],
                            maskt[:, c * MB + (s_lo - alo):c * MB + (s_hi - alo)],
                        )
                        for sg in subs:
                            sl = sg - hq * NSUB
                            at, off, _ = acc_region(acc_tiles, sl)
                            nc.tensor.matmul(
                                at[:, off + 65:off + 130],
                                el[:, sg * 128 - s_lo:sg * 128 - s_lo + 128],
                                vt[c][:],
                                start=False,
                                stop=(c == sub_chunks[sg][-1]),
                                skip_group_check=True,
                            )

                def emit_rescale(hq):
                    acc_tiles, comb, _, rrs = halves[hq]
                    for t, g in enumerate(ACC_GROUPS):
                        w = 130 * len(g)
                        rr = p_rr.tile([128, 6], f32, tag="rr", name=f"rr{hq}_{t}")
                        nc.vector.reciprocal(
                            rr[:, 0:2 * len(g)], acc_tiles[t][:, 64:w:65]
                        )
                        nc.vector.tensor_mul(
                            rr[:, 0:2 * len(g)], rr[:, 0:2 * len(g)],
                            smalls[:, 0:2 * len(g)]
                        )
                        rrs.append(rr)
                    for sl in range(NSUB):
                        at, off, t = acc_region(acc_tiles, sl)
                        rr = rrs[t]
                        kk = sl - (0, 3, 6)[t]
                        t1 = p_t1.tile([128, DH], f32, tag="t1")
                        nc.vector.tensor_scalar_mul(
                            t1[:], at[:, off:off + DH], rr[:, 2 * kk:2 * kk + 1]
                        )
                        nc.vector.scalar_tensor_tensor(
                            comb[:, sl * 128:sl * 128 + DH],
                            at[:, off + 65:off + 65 + DH],
                            rr[:, 2 * kk + 1:2 * kk + 2],
                            t1[:],
                            OP.mult,
                            OP.add,
                        )

                def emit_xbar(hq, sl0, nsl):
                    _, comb, combT, _ = halves[hq]
                    nc.sync.dma_start_transpose(
                        combT[:, sl0 * 128:(sl0 + nsl) * 128].rearrange(
                            "p (j c) -> p j c", j=nsl
                        ),
                        comb[:, sl0 * 128:(sl0 + nsl) * 128],
                    )

                def emit_proj(hq, sl, op_tile, copy_eng):
                    _, _, combT, _ = halves[hq]
                    sg = hq * NSUB + sl
                    nc.tensor.matmul(
                        op_tile[:, 0:D],
                        combT[0:DH, sl * 128:(sl + 1) * 128],
                        wot,
                        start=True,
                        stop=True,
                    )
                    if copy_eng == "act":
                        nc.scalar.copy(outbuf[:, sg * D:(sg + 1) * D], op_tile[:, 0:D])
                    else:
                        nc.vector.tensor_copy(
                            outbuf[:, sg * D:(sg + 1) * D], op_tile[:, 0:D]
                        )

                def emit_outdma(hq, sl0, nsl):
                    # gpsimd (SWDGE) queue: keeps the serialized HWDGE path
                    # free for input DMAs and XBAR transposes
                    g0 = hq * NSUB + sl0
                    dst = out_d[g0 * 128:(g0 + nsl) * 128, :].rearrange(
                        "(j p) c -> p j c", p=128
                    )
                    src = outbuf[:, g0 * D:(g0 + nsl) * D].rearrange(
                        "p (j c) -> p j c", j=nsl
                    )
                    nc.gpsimd.dma_start(dst, src)

                def new_half(hq):
                    acc_tiles = [
                        p_acc.tile([128, 130 * len(g)], f32, tag=f"acc{t}",
                                   name=f"acc{t}_{hq}")
                        for t, g in enumerate(ACC_GROUPS)
                    ]
                    comb = p_comb.tile([128, NSUB * 128], bf16, tag="comb",
                                       name=f"comb{hq}")
                    combT = p_ct.tile([128, NSUB * 128], bf16, tag="ct",
                                      name=f"combT{hq}")
                    nc.gpsimd.memset(comb[:], 0.0)
                    halves.append((acc_tiles, comb, combT, []))
                    return acc_tiles

                # ---- half 0 (projections interleaved with the chunk loop:
                # V runs 2 chunks ahead through the op pool; QK windows 2/3
                # slot into the scores pool once xt panel 1 has landed) ----
                acc0 = new_half(0)
                emit_init(0, acc0)
                qk_window(0)
                qk_window(1)
                v_chunk(0)
                v_chunk(1)
                for c in range(NCHUNK):
                    ec = emit_scores_exp(0, c)
                    emit_av(0, c, ec, acc0)
                    if c + 2 < NCHUNK:
                        v_chunk(c + 2)
                    if c == 0:
                        qk_window(2)
                    if c == 1:
                        qk_window(3)
                emit_rescale(0)

                # ---- half 1, with half-0 projection spread through it ----
                acc1 = new_half(1)
                pend = []  # deferred AV blocks before init (init waits on
                # half-0 accumulator release; keep PE/Act fed meanwhile)
                for c in range(3):
                    pend.append((c, emit_scores_exp(1, c)))
                emit_init(1, acc1)
                for c, ec in pend:
                    emit_av(1, c, ec, acc1)
                for c in range(3, NCHUNK):
                    ec = emit_scores_exp(1, c)
                    emit_av(1, c, ec, acc1)
                    # interleave half-0 projection: one subtile per chunk
                    if c == 3:
                        emit_xbar(0, 0, 4)
                    if c == 7:
                        emit_xbar(0, 4, 4)
                    if 4 <= c <= 11:
                        sl = c - 4
                        op = p_op.tile([128, D], f32, tag="op", name=f"op0_{sl}")
                        emit_proj(0, sl, op, "dve")
                        if sl % 4 == 3:
                            emit_outdma(0, sl - 3, 4)

                # ---- half-1 readout (tail) ----
                emit_rescale(1)
                for sl in range(NSUB):
                    if sl % 2 == 0:
                        emit_xbar(1, sl, 2)
                    # rotate 3 psum buffers: op pool + two score-pool tiles
                    if sl % 3 == 0:
                        op = p_op.tile([128, D], f32, tag="op", name=f"op1_{sl}")
                    else:
                        op = p_s.tile([128, QHALF], f32, tag="s", name=f"ops1_{sl}")
                    emit_proj(1, sl, op, "act" if sl % 2 == 0 else "dve")
                    if sl % 2 == 1:
                        emit_outdma(1, sl - 1, 2)

    _legalize_waits(nc, mybir)
    return nc


_CACHE = {}


def _get_built(bar_key, bars):
    if bar_key not in _CACHE:
        _CACHE[bar_key] = _build(bars)
    return _CACHE[bar_key]


def _np_reference(hidden_states, bar_positions, attention_mask, Wq, bq, Wk, bk,
                  Wv, bv, Wo, bo, bar_emb, gate):
    """Plain numpy fallback (only used if inputs violate baked assumptions)."""
    B, S_, _ = hidden_states.shape
    x = hidden_states.astype(np.float64)
    q = (x @ Wq.T + bq).reshape(B, S_, H, DH).transpose(0, 2, 1, 3)
    k = (x @ Wk.T + bk).reshape(B, S_, H, DH).transpose(0, 2, 1, 3)
    v = (x @ Wv.T + bv).reshape(B, S_, H, DH).transpose(0, 2, 1, 3)
    scores = np.einsum("bhqd,bhkd->bhqk", q, k) * SCALE
    pad = attention_mask[:, None, None, :]
    bar_mask = (bar_positions[:, :, None] == bar_positions[:, None, :])[:, None]
    NEG = -np.inf

    def softmax(s):
        s = s - s.max(-1, keepdims=True)
        e = np.exp(s)
        return e / e.sum(-1, keepdims=True)

    local = softmax(np.where(bar_mask & pad, scores, NEG))
    emb = bar_emb[np.asarray(bar_positions) % bar_emb.shape[0]]
    bias = np.sum(emb * emb, axis=-1)
    glob = softmax(np.where(pad, scores + bias[:, None, :, None], NEG))
    la = np.einsum("bhqk,bhkd->bhqd", local, v)
    ga = np.einsum("bhqk,bhkd->bhqd", glob, v)
    g = 1.0 / (1.0 + np.exp(-gate))[None, :, None, None]
    comb = g * la + (1.0 - g) * ga
    out = comb.transpose(0, 2, 1, 3).reshape(B, S_, H * DH)
    return (out @ Wo.T + bo).astype(np.float32)


def kernel(**inputs):
    import ml_dtypes

    bf = ml_dtypes.bfloat16
    hidden_states = np.asarray(inputs["hidden_states"], dtype=np.float32)
    bar_positions = np.asarray(inputs["bar_positions"])
    attention_mask = np.asarray(inputs["attention_mask"])
    Wq = np.asarray(inputs["Wq"], dtype=np.float32)
    bq = np.asarray(inputs["bq"], dtype=np.float32)
    Wk = np.asarray(inputs["Wk"], dtype=np.float32)
    bk = np.asarray(inputs["bk"], dtype=np.float32)
    Wv = np.asarray(inputs["Wv"], dtype=np.float32)
    bv = np.asarray(inputs["bv"], dtype=np.float32)
    Wo = np.asarray(inputs["Wo"], dtype=np.float32)
    bo = np.asarray(inputs["bo"], dtype=np.float32)
    gate = np.asarray(inputs["gate"], dtype=np.float32)

    bp = bar_positions[0].astype(np.int64)
    if (
        hidden_states.shape != (1, S, D)
        or not bool(attention_mask.all())
        or not bool((np.diff(bp) >= 0).all())
        or np.any(bq)
        or np.any(bk)
    ):
        return _np_reference(
            hidden_states, bar_positions, attention_mask, Wq, bq, Wk, bk,
            Wv, bv, Wo, bo, np.asarray(inputs["bar_emb"], dtype=np.float32), gate,
        )

    bars = _bar_bounds(bp)
    nc = _get_built(bp.tobytes(), bars)

    # aligned mask bands (same for every core)
    aband = _abands(bars)
    maskband = np.zeros((128, NCHUNK * MB), dtype=bf)
    for c in range(NCHUNK):
        alo, ahi = aband[c]
        eq = (bp[c * 128:(c + 1) * 128, None] == bp[None, alo:ahi])
        maskband[:, c * MB:c * MB + (ahi - alo)] = eq.astype(bf)

    xt = np.ascontiguousarray(hidden_states[0].T).astype(bf)  # [512, 2048]
    g = 1.0 / (1.0 + np.exp(-gate.astype(np.float64)))  # sigmoid, [H]
    in_maps = []
    for h in range(H):
        sl = slice(h * DH, (h + 1) * DH)
        wall = np.zeros((128, WALL_W), dtype=np.float32)
        for kc in range(4):
            r = slice(kc * 128, (kc + 1) * 128)
            wall[:, kc * 128:kc * 128 + 64] = Wq[sl, r].T * np.float32(SCALE)
            wall[:, kc * 128 + 64:(kc + 1) * 128] = Wk[sl, r].T
            wall[:, 512 + kc * DH:512 + (kc + 1) * DH] = Wv[sl, r].T
        wall[0:DH, 768:768 + D] = Wo[:, sl].T
        smalls = np.zeros((128, 8), dtype=np.float32)
        smalls[:, 0:6:2] = np.float32(1.0 - g[h])
        smalls[:, 1:6:2] = np.float32(g[h])
        in_maps.append({
            "xt": xt,
            "wall": wall.astype(bf),
            "smalls": smalls,
            "maskband": maskband,
        })

    res = _run_spmd(nc, in_maps)
    out = np.zeros((S, D), dtype=np.float32)
    for h in range(H):
        out += res.results[h]["out_partial"]
    out += bo + Wo @ bv
    return out.reshape(1, S, D)


def _run_spmd(nc, in_maps, **kw):
    from concourse.bass_utils import run_bass_kernel_spmd

    return run_bass_kernel_spmd(nc, in_maps, list(range(H)), **kw)
